# revision 6
# baseline (speedup 1.0000x reference)
"""Self-attention kernel for TRN2, data-parallel over batch (8 cores).

Per core (one batch element):
  x is transposed to xT via fp32 TensorE transposes, quantized to fp8e4 on the
  psum->SBUF copy.  All projections and the attention matmuls (QK^T, PV, out)
  run as fp8e4 DoubleRow matmuls (K=256 per instruction, 2x bf16 throughput).
  Weights are prescaled by 16 to sit in the fp8e4 normal range; the 1/16 is
  folded into the psum epilogues.  Scores are computed transposed (sT[s,t]) so
  the exp output feeds PV directly; exp has bias -5*ln2 so the fp8 attention
  weights stay well below the e4m3 max (240); the scaling cancels in the
  softmax normalization.  Row sums come from a 4.0-column appended to v,
  which exactly absorbs the a-quantization scale (x0.25) and the fp8 out-proj
  weight prescale (x16): y = yps * recip(rowsum) + (x + ba).

  Scheduling: one set of PSUM pools for the whole kernel (no pool barriers).
  The transpose and k/q projection streams are interleaved per t-block so
  projections start as soon as their xT columns land.  The attention is
  software-pipelined at row-tile granularity: a burst of score matmuls for
  block tb+1, then a full 8-matmul PV chain of block tb, its drain, and one
  output-projection tile of block tb-1 - so the Scalar engine's exp hides
  behind PV/output matmuls and matmul chains stay long.  Engine balance:
  q/k epilogues + half the xT copies on Scalar, v/a/y epilogues on Vector,
  weight casts and the residual x+ba on GpSimd.
"""

import numpy as np

import concourse.bass as bass
import concourse.mybir as mybir
import concourse.tile as tile
from concourse import bacc
from concourse.bass import ds, ts
from concourse.bass_utils import run_bass_kernel_spmd
from concourse.masks import make_identity

F32 = mybir.dt.float32
BF16 = mybir.dt.bfloat16
E4 = mybir.dt.float8e4
AF = mybir.ActivationFunctionType
DR = mybir.MatmulPerfMode.DoubleRow
MUL = mybir.AluOpType.mult
ADD = mybir.AluOpType.add

B, T, C, U, P = 8, 2048, 512, 256, 128
TC = T // P    # 16 row tiles
CCH = C // P   # 4 c-chunks (2 DoubleRow pairs)
UCH = U // P   # 2 u-chunks (1 DoubleRow pair)
TBLK = 512     # t-block for attention
NTB = T // TBLK
NPR = TC // 2  # 8 key-chunk pairs
SCALE = 1.0 / float(np.sqrt(U))
NLN2 = -3.4657359027997265  # -5 ln 2: keep exp output well below the fp8e4 max

_cache = {}


def _build_kernel(tc):
    nc = tc.nc
    x = nc.dram_tensor("x", [T, C], F32, kind="ExternalInput").ap()
    Wq = nc.dram_tensor("Wq", [C, U], F32, kind="ExternalInput").ap()
    bq = nc.dram_tensor("bq", [U], F32, kind="ExternalInput").ap()
    Wk = nc.dram_tensor("Wk", [C, U], F32, kind="ExternalInput").ap()
    bk = nc.dram_tensor("bk", [U], F32, kind="ExternalInput").ap()
    Wv = nc.dram_tensor("Wv", [C, U], F32, kind="ExternalInput").ap()
    bv = nc.dram_tensor("bv", [U], F32, kind="ExternalInput").ap()
    Wa = nc.dram_tensor("Wa", [U, C], F32, kind="ExternalInput").ap()
    ba = nc.dram_tensor("ba", [C], F32, kind="ExternalInput").ap()
    out = nc.dram_tensor("out", [T, C], F32, kind="ExternalOutput").ap()

    consts = tc.alloc_tile_pool(name="consts", bufs=1)
    persist = tc.alloc_tile_pool(name="persist", bufs=1)
    # unified PSUM pools (8 banks total, live for the whole kernel)
    spsum = tc.alloc_tile_pool(name="spsum", bufs=2, space="PSUM")  # 2x2 banks
    apsum = tc.alloc_tile_pool(name="apsum", bufs=2, space="PSUM")  # 2x1 bank
    ypsum = tc.alloc_tile_pool(name="ypsum", bufs=2, space="PSUM")  # 2x1 bank
    p_pool = tc.alloc_tile_pool(name="p_pool", bufs=2 * NPR + 2)
    a_pool = tc.alloc_tile_pool(name="a_pool", bufs=4)
    rcp_pool = tc.alloc_tile_pool(name="rcp_pool", bufs=9)
    y_pool = tc.alloc_tile_pool(name="y_pool", bufs=2)

    # --- constants ---
    identity32 = consts.tile([P, P], F32)
    make_identity(nc, identity32)
    identity8 = consts.tile([P, P], E4)
    make_identity(nc, identity8)
    nln2 = consts.tile([P, 1], F32)
    nc.vector.memset(nln2, NLN2)

    # persistent layout tensors
    x_sb = persist.tile([P, TC, C], F32)       # x rows (transpose src)
    xba_sb = persist.tile([P, TC, C], F32)     # x + ba (residual)
    xT8 = persist.tile([P, CCH, T], E4)        # x^T fp8 (c on partitions)
    qT8 = persist.tile([P, UCH, T], E4)        # q^T fp8 (u on partitions)
    kT8 = persist.tile([P, UCH, T], E4)        # k^T fp8
    v8 = persist.tile([P, NPR, 2, U + 8], E4)  # v fp8 + 4.0 col, padded
    aT8 = persist.tile([P, UCH, T], E4)        # a^T fp8 (unnormalized)
    nc.vector.memset(v8[:, :, :, U:U + 1], 4.0)

    # --- input DMAs: x in 8 sub-transfers; weights/biases slotted between
    #     on the sync queue (scalar queue kept light for epilogues) ---
    Wk_st = consts.tile([P, CCH, U], F32)
    Wq_st = consts.tile([P, CCH, U], F32)
    Wv_st = consts.tile([P, CCH, U], F32)
    Wa_st = consts.tile([P, UCH, C], F32)

    def xdma(g, eng):
        eng.dma_start(
            out=x_sb[:, ds(2 * g, 2), :],
            in_=x[ds(256 * g, 256), :].rearrange("(q p) c -> p q c", p=P))

    for g in (1, 3, 5, 7):
        xdma(g, nc.scalar)
    xdma(0, nc.sync)
    xdma(2, nc.sync)
    nc.sync.dma_start(out=Wk_st, in_=Wk.rearrange("(cc p) u -> p cc u", p=P))
    xdma(4, nc.sync)
    nc.sync.dma_start(out=Wq_st, in_=Wq.rearrange("(cc p) u -> p cc u", p=P))
    xdma(6, nc.sync)
    bq_sb = consts.tile([P, UCH], F32)
    nc.sync.dma_start(out=bq_sb, in_=bq.rearrange("(uc p) -> p uc", p=P))
    bk_sb = consts.tile([P, UCH], F32)
    nc.sync.dma_start(out=bk_sb, in_=bk.rearrange("(uc p) -> p uc", p=P))
    bv_row = consts.tile([1, U], F32)
    nc.sync.dma_start(out=bv_row, in_=bv[None, :])
    ba_row = consts.tile([1, C], F32)
    nc.sync.dma_start(out=ba_row, in_=ba[None, :])
    nc.sync.dma_start(out=Wv_st, in_=Wv.rearrange("(cc p) u -> p cc u", p=P))
    nc.sync.dma_start(out=Wa_st, in_=Wa.rearrange("(uc p) c -> p uc c", p=P))

    # fp8 weight casts + bias broadcasts + residual adds on GpSimd
    Wk8 = consts.tile([P, CCH, U], E4)
    nc.gpsimd.tensor_scalar(out=Wk8, in0=Wk_st, scalar1=16.0, scalar2=None, op0=MUL)
    Wq8 = consts.tile([P, CCH, U], E4)
    nc.gpsimd.tensor_scalar(out=Wq8, in0=Wq_st, scalar1=16.0, scalar2=None, op0=MUL)
    bv_full = consts.tile([P, U], F32)
    nc.gpsimd.partition_broadcast(bv_full, bv_row)
    ba_full = consts.tile([P, C], F32)
    nc.gpsimd.partition_broadcast(ba_full, ba_row)
    Wv8 = consts.tile([P, CCH, U], E4)
    nc.gpsimd.tensor_scalar(out=Wv8, in0=Wv_st, scalar1=16.0, scalar2=None, op0=MUL)
    Wa8 = consts.tile([P, UCH, C], E4)
    nc.gpsimd.tensor_scalar(out=Wa8, in0=Wa_st, scalar1=16.0, scalar2=None, op0=MUL)
    for tt in range(TC):
        nc.gpsimd.tensor_tensor(out=xba_sb[:, tt, :], in0=x_sb[:, tt, :],
                                in1=ba_full, op=ADD)

    # warmup (p-state ramp) using a ypsum slot
    wtile = ypsum.tile([P, P], F32, tag="yps", name="warmup")
    for i in range(30):
        nc.tensor.matmul(wtile, lhsT=identity8, rhs=identity8,
                         start=(i == 0), stop=(i == 29))

    # --- phase 1+2 interleaved per t-block: transposes of 4 row tiles, then
    #     the k and q projections of that t-block (fp8 DR, Scalar epilogue) ---
    def proj(W8, bias_sb, dst, tb):
        sps = spsum.tile([P, 2, TBLK], F32, tag="sps")
        for uc in range(UCH):
            for pr in range(CCH // 2):
                nc.tensor.matmul(
                    sps[:, uc, :],
                    lhsT=W8[:, 2 * pr:2 * pr + 2, ts(uc, P)],
                    rhs=xT8[:, 2 * pr:2 * pr + 2, ds(tb * TBLK, TBLK)],
                    start=(pr == 0),
                    stop=(pr == CCH // 2 - 1),
                    perf_mode=DR,
                )
            nc.scalar.activation(
                out=dst[:, uc, ds(tb * TBLK, TBLK)], in_=sps[:, uc, :],
                func=AF.Identity, bias=bias_sb[:, uc:uc + 1],
                scale=1.0 / 16.0,
            )

    for tb in range(NTB):
        for tt in range(4 * tb, 4 * tb + 4):
            tps = ypsum.tile([P, CCH, P], F32, tag="yps")
            for cc in range(CCH):
                nc.tensor.transpose(tps[:, cc, :], x_sb[:, tt, ts(cc, P)],
                                    identity32)
            if tt % 2 == 0:
                nc.vector.tensor_copy(out=xT8[:, :, ts(tt, P)], in_=tps)
            else:
                nc.scalar.activation(out=xT8[:, :, ts(tt, P)], in_=tps,
                                     func=AF.Copy, scale=1.0)
        proj(Wk8, bk_sb, kT8, tb)
        proj(Wq8, bq_sb, qT8, tb)

    pts_by_tb = {}
    rcps_by_tb = {}
    y_blk_by_tb = {}

    def emit_score_pair(tb, spr):
        sps = spsum.tile([P, 2, TBLK], F32, tag="sps")
        for j in range(2):
            nc.tensor.matmul(
                sps[:, j, :],
                lhsT=kT8[:, :, ts(2 * spr + j, P)],
                rhs=qT8[:, :, ds(tb * TBLK, TBLK)],
                start=True,
                stop=True,
                perf_mode=DR,
            )
        pt = p_pool.tile([P, 2, TBLK], E4, tag="pt")
        nc.scalar.activation(out=pt, in_=sps, func=AF.Exp,
                             bias=nln2, scale=SCALE)
        pts_by_tb.setdefault(tb, []).append(pt)

    def emit_v_tiles(tt0, n):
        for tt in range(tt0, tt0 + n):
            aps = apsum.tile([P, 512], F32, tag="aps")
            for pr in range(CCH // 2):
                nc.tensor.matmul(
                    aps[:, 0:U],
                    lhsT=xT8[:, 2 * pr:2 * pr + 2, ts(tt, P)],
                    rhs=Wv8[:, 2 * pr:2 * pr + 2, :],
                    start=(pr == 0),
                    stop=(pr == CCH // 2 - 1),
                    perf_mode=DR,
                )
            nc.vector.scalar_tensor_tensor(
                out=v8[:, tt // 2, tt % 2, 0:U], in0=aps[:, 0:U],
                scalar=1.0 / 16.0, in1=bv_full, op0=MUL, op1=ADD,
            )

    def emit_pv(tb, tsl):
        aps = apsum.tile([P, 512], F32, tag="aps", name=f"aps{tb}_{tsl}")
        for pr in range(NPR):
            nc.tensor.matmul(
                aps[:, 0:U + 1],
                lhsT=pts_by_tb[tb][pr][:, :, ts(tsl, P)],
                rhs=v8[:, pr, :, 0:U + 1],
                start=(pr == 0),
                stop=(pr == NPR - 1),
                perf_mode=DR,
            )
        return aps

    def emit_drain(tb, tsl, aps):
        rcp = rcp_pool.tile([P, 1], F32, tag="rcp")
        nc.vector.reciprocal(rcp, aps[:, U:U + 1])
        rcps_by_tb.setdefault(tb, []).append(rcp)
        a8 = a_pool.tile([P, U], E4, tag="a8")
        nc.vector.tensor_scalar(out=a8, in0=aps[:, 0:U], scalar1=0.25,
                                scalar2=None, op0=MUL)
        # fp8 transpose needs output element step 2 in psum
        tps2 = ypsum.tile([P, UCH, P, 2], E4, tag="yps")
        for uc in range(UCH):
            nc.tensor.transpose(tps2[:, uc, :, 0:1], a8[:, ts(uc, P)],
                                identity8)
        nc.vector.tensor_copy(
            out=aT8[:, :, ds(tb * TBLK + tsl * P, P)],
            in_=tps2[:, :, :, 0])

    def emit_finish_one(tb, tsl):
        if tsl == 0:
            y_blk_by_tb[tb] = y_pool.tile([P, NTB, C], F32, tag="ysb",
                                          name=f"yblk{tb}")
        yps = ypsum.tile([P, C], F32, tag="yps")
        nc.tensor.matmul(
            yps,
            lhsT=aT8[:, :, ds(tb * TBLK + tsl * P, P)],
            rhs=Wa8,
            start=True,
            stop=True,
            perf_mode=DR,
        )
        nc.vector.scalar_tensor_tensor(
            out=y_blk_by_tb[tb][:, tsl, :], in0=yps, scalar=rcps_by_tb[tb][tsl],
            in1=xba_sb[:, tb * NTB + tsl, :], op0=MUL, op1=ADD,
        )
        if tsl == NTB - 1:
            nc.sync.dma_start(
                out=out[ds(tb * TBLK, TBLK), :].rearrange(
                    "(q p) c -> p q c", p=P),
                in_=y_blk_by_tb.pop(tb))

    # scores for tb0 interleaved with the v projection
    for spr in range(NPR):
        emit_score_pair(0, spr)
        emit_v_tiles(2 * spr, 2)

    # --- phase 3: pipelined attention, row-tile bursts ---
    for tb in range(NTB):
        for tsl in range(NTB):
            if tb + 1 < NTB:
                emit_score_pair(tb + 1, 2 * tsl)
                emit_score_pair(tb + 1, 2 * tsl + 1)
            aps = emit_pv(tb, tsl)
            emit_drain(tb, tsl, aps)
            if tb >= 1:
                emit_finish_one(tb - 1, tsl)
            if tb == NTB - 1 and tsl >= 1:
                emit_finish_one(tb, tsl - 1)
        pts_by_tb.pop(tb)
    emit_finish_one(NTB - 1, NTB - 1)

    for pool in (y_pool, rcp_pool, a_pool, p_pool,
                 ypsum, apsum, spsum, persist, consts):
        pool.release()


def _get_nc():
    if "nc" not in _cache:
        nc = bacc.Bacc("TRN2", target_bir_lowering=False, debug=False)
        with tile.TileContext(nc) as tc:
            _build_kernel(tc)
        nc.compile()
        _cache["nc"] = nc
    return _cache["nc"]


def kernel(**inputs):
    nc = _get_nc()
    shared = {k: np.ascontiguousarray(np.asarray(v, dtype=np.float32))
              for k, v in inputs.items() if k != "x"}
    xs = np.ascontiguousarray(np.asarray(inputs["x"], dtype=np.float32))
    in_maps = [dict(shared, x=xs[b]) for b in range(B)]
    res = run_bass_kernel_spmd(nc, in_maps, core_ids=list(range(B)))
    return np.stack([res.results[b]["out"] for b in range(B)], axis=0)


# revision 7
# speedup vs baseline: 1.7272x; 1.7272x over previous
"""Self-attention kernel for TRN2, data-parallel over batch (8 cores).

Per core (one batch element):
  x is transposed to xT via fp32 TensorE transposes, quantized to fp8e4 on the
  psum->SBUF copy.  All projections and the attention matmuls (QK^T, PV, out)
  run as fp8e4 DoubleRow matmuls (K=256 per instruction, 2x bf16 throughput).
  Weights are prescaled by 16 to sit in the fp8e4 normal range; the 1/16 is
  folded into the psum epilogues.  Scores are computed transposed (sT[s,t]) so
  the exp output feeds PV directly; exp has bias -5*ln2 so the fp8 attention
  weights stay well below the e4m3 max (240); the scaling cancels in the
  softmax normalization.  Row sums come from a 4.0-column appended to v,
  which exactly absorbs the a-quantization scale (x0.25) and the fp8 out-proj
  weight prescale (x16): y = yps * recip(rowsum) + (x + ba).

  Scheduling: one set of PSUM pools for the whole kernel (no pool barriers).
  The transpose and k/q projection streams are interleaved per t-block so
  projections start as soon as their xT columns land.  The attention is
  software-pipelined at row-tile granularity: a burst of score matmuls for
  block tb+1, then a full 8-matmul PV chain of block tb, its drain, and one
  output-projection tile of block tb-1 - so the Scalar engine's exp hides
  behind PV/output matmuls and matmul chains stay long.  Engine balance:
  q/k epilogues + half the xT copies on Scalar, v/a/y epilogues on Vector,
  weight casts and the residual x+ba on GpSimd.
"""

import numpy as np

import concourse.bass as bass
import concourse.mybir as mybir
import concourse.tile as tile
from concourse import bacc
from concourse.bass import ds, ts
from concourse.bass_utils import run_bass_kernel_spmd
from concourse.masks import make_identity

F32 = mybir.dt.float32
BF16 = mybir.dt.bfloat16
E4 = mybir.dt.float8e4
AF = mybir.ActivationFunctionType
DR = mybir.MatmulPerfMode.DoubleRow
MUL = mybir.AluOpType.mult
ADD = mybir.AluOpType.add

B, T, C, U, P = 8, 2048, 512, 256, 128
TC = T // P    # 16 row tiles
CCH = C // P   # 4 c-chunks (2 DoubleRow pairs)
UCH = U // P   # 2 u-chunks (1 DoubleRow pair)
TBLK = 512     # t-block for attention
NTB = T // TBLK
NPR = TC // 2  # 8 key-chunk pairs
SCALE = 1.0 / float(np.sqrt(U))
NLN2 = -3.4657359027997265  # -5 ln 2: keep exp output well below the fp8e4 max

_cache = {}


def _build_kernel(tc):
    nc = tc.nc
    x = nc.dram_tensor("x", [T, C], F32, kind="ExternalInput").ap()
    Wq = nc.dram_tensor("Wq", [C, U], F32, kind="ExternalInput").ap()
    bq = nc.dram_tensor("bq", [U], F32, kind="ExternalInput").ap()
    Wk = nc.dram_tensor("Wk", [C, U], F32, kind="ExternalInput").ap()
    bk = nc.dram_tensor("bk", [U], F32, kind="ExternalInput").ap()
    Wv = nc.dram_tensor("Wv", [C, U], F32, kind="ExternalInput").ap()
    bv = nc.dram_tensor("bv", [U], F32, kind="ExternalInput").ap()
    Wa = nc.dram_tensor("Wa", [U, C], F32, kind="ExternalInput").ap()
    ba = nc.dram_tensor("ba", [C], F32, kind="ExternalInput").ap()
    out = nc.dram_tensor("out", [T, C], F32, kind="ExternalOutput").ap()

    consts = tc.alloc_tile_pool(name="consts", bufs=1)
    persist = tc.alloc_tile_pool(name="persist", bufs=1)
    # unified PSUM pools (8 banks total, live for the whole kernel)
    spsum = tc.alloc_tile_pool(name="spsum", bufs=2, space="PSUM")  # 2x2 banks
    apsum = tc.alloc_tile_pool(name="apsum", bufs=2, space="PSUM")  # 2x1 bank
    ypsum = tc.alloc_tile_pool(name="ypsum", bufs=2, space="PSUM")  # 2x1 bank
    p_pool = tc.alloc_tile_pool(name="p_pool", bufs=2 * NPR + 2)
    a_pool = tc.alloc_tile_pool(name="a_pool", bufs=4)
    rcp_pool = tc.alloc_tile_pool(name="rcp_pool", bufs=9)
    y_pool = tc.alloc_tile_pool(name="y_pool", bufs=2)

    # --- constants ---
    identity32 = consts.tile([P, P], F32)
    make_identity(nc, identity32)
    identity8 = consts.tile([P, P], E4)
    make_identity(nc, identity8)
    nln2 = consts.tile([P, 1], F32)
    nc.vector.memset(nln2, NLN2)

    # persistent layout tensors
    x_sb = persist.tile([P, TC, C], F32)       # x rows (transpose src)
    xba_sb = persist.tile([P, TC, C], F32)     # x + ba (residual)
    xT8 = persist.tile([P, CCH, T], E4)        # x^T fp8 (c on partitions)
    qT8 = persist.tile([P, UCH, T], E4)        # q^T fp8 (u on partitions)
    kT8 = persist.tile([P, UCH, T], E4)        # k^T fp8
    v8 = persist.tile([P, NPR, 2, U + 8], E4)  # v fp8 + 4.0 col, padded
    aT8 = persist.tile([P, UCH, T], E4)        # a^T fp8 (unnormalized)
    nc.vector.memset(v8[:, :, :, U:U + 1], 4.0)

    # --- input DMAs: x in 8 sub-transfers; weights/biases slotted between
    #     on the sync queue (scalar queue kept light for epilogues) ---
    Wk_st = consts.tile([P, CCH, U], F32)
    Wq_st = consts.tile([P, CCH, U], F32)
    Wv_st = consts.tile([P, CCH, U], F32)
    Wa_st = consts.tile([P, UCH, C], F32)

    def xdma(g, eng):
        eng.dma_start(
            out=x_sb[:, ds(2 * g, 2), :],
            in_=x[ds(256 * g, 256), :].rearrange("(q p) c -> p q c", p=P))

    for g in (1, 3, 5, 7):
        xdma(g, nc.scalar)
    xdma(0, nc.sync)
    xdma(2, nc.sync)
    nc.sync.dma_start(out=Wk_st, in_=Wk.rearrange("(cc p) u -> p cc u", p=P))
    xdma(4, nc.sync)
    nc.sync.dma_start(out=Wq_st, in_=Wq.rearrange("(cc p) u -> p cc u", p=P))
    xdma(6, nc.sync)
    bq_sb = consts.tile([P, UCH], F32)
    nc.sync.dma_start(out=bq_sb, in_=bq.rearrange("(uc p) -> p uc", p=P))
    bk_sb = consts.tile([P, UCH], F32)
    nc.sync.dma_start(out=bk_sb, in_=bk.rearrange("(uc p) -> p uc", p=P))
    bv_row = consts.tile([1, U], F32)
    nc.sync.dma_start(out=bv_row, in_=bv[None, :])
    ba_row = consts.tile([1, C], F32)
    nc.sync.dma_start(out=ba_row, in_=ba[None, :])
    nc.sync.dma_start(out=Wv_st, in_=Wv.rearrange("(cc p) u -> p cc u", p=P))
    nc.sync.dma_start(out=Wa_st, in_=Wa.rearrange("(uc p) c -> p uc c", p=P))

    # fp8 weight casts: k/v/a on DVE, q on Scalar (ACT copy with x16 scale);
    # bias broadcasts + residual adds on GpSimd
    Wk8 = consts.tile([P, CCH, U], E4)
    nc.vector.tensor_scalar(out=Wk8, in0=Wk_st, scalar1=16.0, scalar2=None, op0=MUL)
    Wq8 = consts.tile([P, CCH, U], E4)
    nc.scalar.activation(out=Wq8, in_=Wq_st, func=AF.Copy, scale=16.0)
    bv_full = consts.tile([P, U], F32)
    nc.gpsimd.partition_broadcast(bv_full, bv_row)
    ba_full = consts.tile([P, C], F32)
    nc.gpsimd.partition_broadcast(ba_full, ba_row)
    Wv8 = consts.tile([P, CCH, U], E4)
    nc.vector.tensor_scalar(out=Wv8, in0=Wv_st, scalar1=16.0, scalar2=None, op0=MUL)
    Wa8 = consts.tile([P, UCH, C], E4)
    nc.vector.tensor_scalar(out=Wa8, in0=Wa_st, scalar1=16.0, scalar2=None, op0=MUL)
    for tt in range(TC):
        nc.gpsimd.tensor_tensor(out=xba_sb[:, tt, :], in0=x_sb[:, tt, :],
                                in1=ba_full, op=ADD)

    # warmup (p-state ramp) using a ypsum slot
    wtile = ypsum.tile([P, P], F32, tag="yps", name="warmup")
    for i in range(30):
        nc.tensor.matmul(wtile, lhsT=identity8, rhs=identity8,
                         start=(i == 0), stop=(i == 29))

    # --- phase 1+2 interleaved per t-block: transposes of 4 row tiles, then
    #     the k and q projections of that t-block (fp8 DR, Scalar epilogue) ---
    def proj(W8, bias_sb, dst, tb):
        sps = spsum.tile([P, 2, TBLK], F32, tag="sps")
        for uc in range(UCH):
            for pr in range(CCH // 2):
                nc.tensor.matmul(
                    sps[:, uc, :],
                    lhsT=W8[:, 2 * pr:2 * pr + 2, ts(uc, P)],
                    rhs=xT8[:, 2 * pr:2 * pr + 2, ds(tb * TBLK, TBLK)],
                    start=(pr == 0),
                    stop=(pr == CCH // 2 - 1),
                    perf_mode=DR,
                )
            nc.scalar.activation(
                out=dst[:, uc, ds(tb * TBLK, TBLK)], in_=sps[:, uc, :],
                func=AF.Identity, bias=bias_sb[:, uc:uc + 1],
                scale=1.0 / 16.0,
            )

    for tb in range(NTB):
        for tt in range(4 * tb, 4 * tb + 4):
            tps = ypsum.tile([P, CCH, P], F32, tag="yps")
            for cc in range(CCH):
                nc.tensor.transpose(tps[:, cc, :], x_sb[:, tt, ts(cc, P)],
                                    identity32)
            if tt % 2 == 0:
                nc.vector.tensor_copy(out=xT8[:, :, ts(tt, P)], in_=tps)
            else:
                nc.scalar.activation(out=xT8[:, :, ts(tt, P)], in_=tps,
                                     func=AF.Copy, scale=1.0)
        proj(Wk8, bk_sb, kT8, tb)
        proj(Wq8, bq_sb, qT8, tb)

    pts_by_tb = {}
    rcps_by_tb = {}
    y_blk_by_tb = {}

    def emit_score_pair(tb, spr):
        sps = spsum.tile([P, 2, TBLK], F32, tag="sps")
        for j in range(2):
            nc.tensor.matmul(
                sps[:, j, :],
                lhsT=kT8[:, :, ts(2 * spr + j, P)],
                rhs=qT8[:, :, ds(tb * TBLK, TBLK)],
                start=True,
                stop=True,
                perf_mode=DR,
            )
        pt = p_pool.tile([P, 2, TBLK], E4, tag="pt")
        nc.scalar.activation(out=pt, in_=sps, func=AF.Exp,
                             bias=nln2, scale=SCALE)
        pts_by_tb.setdefault(tb, []).append(pt)

    def emit_v_tiles(tt0, n):
        for tt in range(tt0, tt0 + n):
            aps = apsum.tile([P, 512], F32, tag="aps")
            for pr in range(CCH // 2):
                nc.tensor.matmul(
                    aps[:, 0:U],
                    lhsT=xT8[:, 2 * pr:2 * pr + 2, ts(tt, P)],
                    rhs=Wv8[:, 2 * pr:2 * pr + 2, :],
                    start=(pr == 0),
                    stop=(pr == CCH // 2 - 1),
                    perf_mode=DR,
                )
            nc.vector.scalar_tensor_tensor(
                out=v8[:, tt // 2, tt % 2, 0:U], in0=aps[:, 0:U],
                scalar=1.0 / 16.0, in1=bv_full, op0=MUL, op1=ADD,
            )

    def emit_pv(tb, tsl):
        aps = apsum.tile([P, 512], F32, tag="aps", name=f"aps{tb}_{tsl}")
        for pr in range(NPR):
            nc.tensor.matmul(
                aps[:, 0:U + 1],
                lhsT=pts_by_tb[tb][pr][:, :, ts(tsl, P)],
                rhs=v8[:, pr, :, 0:U + 1],
                start=(pr == 0),
                stop=(pr == NPR - 1),
                perf_mode=DR,
            )
        return aps

    def emit_drain(tb, tsl, aps):
        rcp = rcp_pool.tile([P, 1], F32, tag="rcp")
        nc.vector.reciprocal(rcp, aps[:, U:U + 1])
        rcps_by_tb.setdefault(tb, []).append(rcp)
        a8 = a_pool.tile([P, U], E4, tag="a8")
        nc.vector.tensor_scalar(out=a8, in0=aps[:, 0:U], scalar1=0.25,
                                scalar2=None, op0=MUL)
        # fp8 transpose needs output element step 2 in psum
        tps2 = ypsum.tile([P, UCH, P, 2], E4, tag="yps")
        for uc in range(UCH):
            nc.tensor.transpose(tps2[:, uc, :, 0:1], a8[:, ts(uc, P)],
                                identity8)
        nc.vector.tensor_copy(
            out=aT8[:, :, ds(tb * TBLK + tsl * P, P)],
            in_=tps2[:, :, :, 0])

    def emit_finish_one(tb, tsl):
        if tsl == 0:
            y_blk_by_tb[tb] = y_pool.tile([P, NTB, C], F32, tag="ysb",
                                          name=f"yblk{tb}")
        yps = ypsum.tile([P, C], F32, tag="yps")
        nc.tensor.matmul(
            yps,
            lhsT=aT8[:, :, ds(tb * TBLK + tsl * P, P)],
            rhs=Wa8,
            start=True,
            stop=True,
            perf_mode=DR,
        )
        nc.vector.scalar_tensor_tensor(
            out=y_blk_by_tb[tb][:, tsl, :], in0=yps, scalar=rcps_by_tb[tb][tsl],
            in1=xba_sb[:, tb * NTB + tsl, :], op0=MUL, op1=ADD,
        )
        if tsl == NTB - 1:
            nc.sync.dma_start(
                out=out[ds(tb * TBLK, TBLK), :].rearrange(
                    "(q p) c -> p q c", p=P),
                in_=y_blk_by_tb.pop(tb))

    # scores for tb0 interleaved with the v projection
    for spr in range(NPR):
        emit_score_pair(0, spr)
        emit_v_tiles(2 * spr, 2)

    # --- phase 3: pipelined attention, row-tile bursts ---
    for tb in range(NTB):
        for tsl in range(NTB):
            if tb + 1 < NTB:
                emit_score_pair(tb + 1, 2 * tsl)
                emit_score_pair(tb + 1, 2 * tsl + 1)
            aps = emit_pv(tb, tsl)
            emit_drain(tb, tsl, aps)
            if tb >= 1:
                emit_finish_one(tb - 1, tsl)
            if tb == NTB - 1 and tsl >= 1:
                emit_finish_one(tb, tsl - 1)
        pts_by_tb.pop(tb)
    emit_finish_one(NTB - 1, NTB - 1)

    for pool in (y_pool, rcp_pool, a_pool, p_pool,
                 ypsum, apsum, spsum, persist, consts):
        pool.release()


def _get_nc():
    if "nc" not in _cache:
        nc = bacc.Bacc("TRN2", target_bir_lowering=False, debug=False)
        with tile.TileContext(nc) as tc:
            _build_kernel(tc)
        nc.compile()
        _cache["nc"] = nc
    return _cache["nc"]


def kernel(**inputs):
    nc = _get_nc()
    shared = {k: np.ascontiguousarray(np.asarray(v, dtype=np.float32))
              for k, v in inputs.items() if k != "x"}
    xs = np.ascontiguousarray(np.asarray(inputs["x"], dtype=np.float32))
    in_maps = [dict(shared, x=xs[b]) for b in range(B)]
    res = run_bass_kernel_spmd(nc, in_maps, core_ids=list(range(B)))
    return np.stack([res.results[b]["out"] for b in range(B)], axis=0)


# revision 8
# speedup vs baseline: 1.8886x; 1.0935x over previous
"""Self-attention kernel for TRN2, data-parallel over batch (8 cores).

Per core (one batch element):
  x is transposed to xT via fp32 TensorE transposes, quantized to fp8e4 on the
  psum->SBUF copy.  All projections and the attention matmuls (QK^T, PV, out)
  run as fp8e4 DoubleRow matmuls (K=256 per instruction, 2x bf16 throughput).
  Weights are prescaled by 16 to sit in the fp8e4 normal range; the 1/16 is
  folded into the psum epilogues.  Scores are computed transposed (sT[s,t]) so
  the exp output feeds PV directly; exp has bias -5*ln2 so the fp8 attention
  weights stay well below the e4m3 max (240); the scaling cancels in the
  softmax normalization.  Row sums come from a 4.0-column appended to v,
  which exactly absorbs the a-quantization scale (x0.25) and the fp8 out-proj
  weight prescale (x16): y = yps * recip(rowsum) + (x + ba).

  Scheduling: one set of PSUM pools for the whole kernel (no pool barriers).
  The transpose and k/q projection streams are interleaved per t-block so
  projections start as soon as their xT columns land.  The attention is
  software-pipelined at row-tile granularity: a burst of score matmuls for
  block tb+1, then a full 8-matmul PV chain of block tb, its drain, and one
  output-projection tile of block tb-1 - so the Scalar engine's exp hides
  behind PV/output matmuls and matmul chains stay long.  Engine balance:
  q/k epilogues + half the xT copies on Scalar, v/a/y epilogues on Vector,
  weight casts and the residual x+ba on GpSimd.
"""

import numpy as np

import concourse.bass as bass
import concourse.mybir as mybir
import concourse.tile as tile
from concourse import bacc
from concourse.bass import ds, ts
from concourse.bass_utils import run_bass_kernel_spmd
from concourse.masks import make_identity

F32 = mybir.dt.float32
BF16 = mybir.dt.bfloat16
E4 = mybir.dt.float8e4
AF = mybir.ActivationFunctionType
DR = mybir.MatmulPerfMode.DoubleRow
MUL = mybir.AluOpType.mult
ADD = mybir.AluOpType.add

B, T, C, U, P = 8, 2048, 512, 256, 128
TC = T // P    # 16 row tiles
CCH = C // P   # 4 c-chunks (2 DoubleRow pairs)
UCH = U // P   # 2 u-chunks (1 DoubleRow pair)
TBLK = 512     # t-block for attention
NTB = T // TBLK
NPR = TC // 2  # 8 key-chunk pairs
SCALE = 1.0 / float(np.sqrt(U))
NLN2 = -3.4657359027997265  # -5 ln 2: keep exp output well below the fp8e4 max

_cache = {}


def _build_kernel(tc):
    nc = tc.nc
    x = nc.dram_tensor("x", [T, C], F32, kind="ExternalInput").ap()
    Wq = nc.dram_tensor("Wq", [C, U], F32, kind="ExternalInput").ap()
    bq = nc.dram_tensor("bq", [U], F32, kind="ExternalInput").ap()
    Wk = nc.dram_tensor("Wk", [C, U], F32, kind="ExternalInput").ap()
    bk = nc.dram_tensor("bk", [U], F32, kind="ExternalInput").ap()
    Wv = nc.dram_tensor("Wv", [C, U], F32, kind="ExternalInput").ap()
    bv = nc.dram_tensor("bv", [U], F32, kind="ExternalInput").ap()
    Wa = nc.dram_tensor("Wa", [U, C], F32, kind="ExternalInput").ap()
    ba = nc.dram_tensor("ba", [C], F32, kind="ExternalInput").ap()
    out = nc.dram_tensor("out", [T, C], F32, kind="ExternalOutput").ap()

    consts = tc.alloc_tile_pool(name="consts", bufs=1)
    persist = tc.alloc_tile_pool(name="persist", bufs=1)
    # unified PSUM pools (8 banks total, live for the whole kernel)
    spsum = tc.alloc_tile_pool(name="spsum", bufs=2, space="PSUM")  # 2x2 banks
    apsum = tc.alloc_tile_pool(name="apsum", bufs=2, space="PSUM")  # 2x1 bank
    ypsum = tc.alloc_tile_pool(name="ypsum", bufs=2, space="PSUM")  # 2x1 bank
    p_pool = tc.alloc_tile_pool(name="p_pool", bufs=2 * NPR + 2)
    a_pool = tc.alloc_tile_pool(name="a_pool", bufs=6)
    rcp_pool = tc.alloc_tile_pool(name="rcp_pool", bufs=9)
    y_pool = tc.alloc_tile_pool(name="y_pool", bufs=2)

    # --- constants ---
    identity32 = consts.tile([P, P], F32)
    make_identity(nc, identity32)
    identity8 = consts.tile([P, P], E4)
    make_identity(nc, identity8)
    nln2 = consts.tile([P, 1], F32)
    nc.vector.memset(nln2, NLN2)

    # persistent layout tensors
    x_sb = persist.tile([P, TC, C], F32)       # x rows (transpose src)
    xba_sb = persist.tile([P, TC, C], F32)     # x + ba (residual)
    xT8 = persist.tile([P, CCH, T], E4)        # x^T fp8 (c on partitions)
    qT8 = persist.tile([P, UCH, T], E4)        # q^T fp8 (u on partitions)
    kT8 = persist.tile([P, UCH, T], E4)        # k^T fp8
    v8 = persist.tile([P, NPR, 2, U + 8], E4)  # v fp8 + 4.0 col, padded
    aT8 = persist.tile([P, UCH, T], E4)        # a^T fp8 (unnormalized)
    nc.vector.memset(v8[:, :, :, U:U + 1], 4.0)

    # --- input DMAs: x in 8 sub-transfers; weights/biases slotted between
    #     on the sync queue (scalar queue kept light for epilogues) ---
    Wk_st = consts.tile([P, CCH, U], F32)
    Wq_st = consts.tile([P, CCH, U], F32)
    Wv_st = consts.tile([P, CCH, U], F32)
    Wa_st = consts.tile([P, UCH, C], F32)

    def xdma(g, eng):
        eng.dma_start(
            out=x_sb[:, ds(2 * g, 2), :],
            in_=x[ds(256 * g, 256), :].rearrange("(q p) c -> p q c", p=P))

    xdma(1, nc.scalar)
    xdma(0, nc.sync)
    nc.sync.dma_start(out=Wk_st, in_=Wk.rearrange("(cc p) u -> p cc u", p=P))
    xdma(3, nc.scalar)
    xdma(2, nc.sync)
    nc.scalar.dma_start(out=Wq_st, in_=Wq.rearrange("(cc p) u -> p cc u", p=P))
    xdma(4, nc.sync)
    xdma(5, nc.scalar)
    xdma(6, nc.sync)
    xdma(7, nc.scalar)
    bq_sb = consts.tile([P, UCH], F32)
    nc.sync.dma_start(out=bq_sb, in_=bq.rearrange("(uc p) -> p uc", p=P))
    bk_sb = consts.tile([P, UCH], F32)
    nc.sync.dma_start(out=bk_sb, in_=bk.rearrange("(uc p) -> p uc", p=P))
    bv_row = consts.tile([1, U], F32)
    nc.sync.dma_start(out=bv_row, in_=bv[None, :])
    ba_row = consts.tile([1, C], F32)
    nc.sync.dma_start(out=ba_row, in_=ba[None, :])
    nc.sync.dma_start(out=Wv_st, in_=Wv.rearrange("(cc p) u -> p cc u", p=P))
    nc.scalar.dma_start(out=Wa_st, in_=Wa.rearrange("(uc p) c -> p uc c", p=P))

    # fp8 weight casts: k/v/a on DVE, q on Scalar (ACT copy with x16 scale);
    # bias broadcasts + residual adds on GpSimd
    Wk8 = consts.tile([P, CCH, U], E4)
    nc.vector.tensor_scalar(out=Wk8, in0=Wk_st, scalar1=16.0, scalar2=None, op0=MUL)
    Wq8 = consts.tile([P, CCH, U], E4)
    nc.scalar.activation(out=Wq8, in_=Wq_st, func=AF.Copy, scale=16.0)
    bv_full = consts.tile([P, U], F32)
    nc.gpsimd.partition_broadcast(bv_full, bv_row)
    ba_full = consts.tile([P, C], F32)
    nc.gpsimd.partition_broadcast(ba_full, ba_row)
    Wv8 = consts.tile([P, CCH, U], E4)
    nc.vector.tensor_scalar(out=Wv8, in0=Wv_st, scalar1=16.0, scalar2=None, op0=MUL)
    Wa8 = consts.tile([P, UCH, C], E4)
    nc.vector.tensor_scalar(out=Wa8, in0=Wa_st, scalar1=16.0, scalar2=None, op0=MUL)
    for tt in range(TC):
        nc.gpsimd.tensor_tensor(out=xba_sb[:, tt, :], in0=x_sb[:, tt, :],
                                in1=ba_full, op=ADD)

    # warmup (p-state ramp) using a ypsum slot
    wtile = ypsum.tile([P, P], F32, tag="yps", name="warmup")
    for i in range(30):
        nc.tensor.matmul(wtile, lhsT=identity8, rhs=identity8,
                         start=(i == 0), stop=(i == 29))

    # --- phase 1+2 interleaved per t-block: transposes of 4 row tiles, then
    #     the k and q projections of that t-block (fp8 DR, Scalar epilogue) ---
    def proj(W8, bias_sb, dst, tb):
        sps = spsum.tile([P, 2, TBLK], F32, tag="sps")
        for uc in range(UCH):
            for pr in range(CCH // 2):
                nc.tensor.matmul(
                    sps[:, uc, :],
                    lhsT=W8[:, 2 * pr:2 * pr + 2, ts(uc, P)],
                    rhs=xT8[:, 2 * pr:2 * pr + 2, ds(tb * TBLK, TBLK)],
                    start=(pr == 0),
                    stop=(pr == CCH // 2 - 1),
                    perf_mode=DR,
                )
            nc.scalar.activation(
                out=dst[:, uc, ds(tb * TBLK, TBLK)], in_=sps[:, uc, :],
                func=AF.Identity, bias=bias_sb[:, uc:uc + 1],
                scale=1.0 / 16.0,
            )

    for tb in range(NTB):
        for tt in range(4 * tb, 4 * tb + 4):
            tps = ypsum.tile([P, CCH, P], F32, tag="yps")
            for cc in range(CCH):
                nc.tensor.transpose(tps[:, cc, :], x_sb[:, tt, ts(cc, P)],
                                    identity32)
            nc.vector.tensor_copy(out=xT8[:, :, ts(tt, P)], in_=tps)
        proj(Wk8, bk_sb, kT8, tb)
        proj(Wq8, bq_sb, qT8, tb)

    pts_by_tb = {}
    rcps_by_tb = {}
    y_blk_by_tb = {}

    def emit_score_pair(tb, spr):
        sps = spsum.tile([P, 2, TBLK], F32, tag="sps")
        for j in range(2):
            nc.tensor.matmul(
                sps[:, j, :],
                lhsT=kT8[:, :, ts(2 * spr + j, P)],
                rhs=qT8[:, :, ds(tb * TBLK, TBLK)],
                start=True,
                stop=True,
                perf_mode=DR,
            )
        pt = p_pool.tile([P, 2, TBLK], E4, tag="pt")
        nc.scalar.activation(out=pt, in_=sps, func=AF.Exp,
                             bias=nln2, scale=SCALE)
        pts_by_tb.setdefault(tb, []).append(pt)

    def emit_v_tiles(tt0, n):
        for tt in range(tt0, tt0 + n):
            aps = apsum.tile([P, 512], F32, tag="aps")
            for pr in range(CCH // 2):
                nc.tensor.matmul(
                    aps[:, 0:U],
                    lhsT=xT8[:, 2 * pr:2 * pr + 2, ts(tt, P)],
                    rhs=Wv8[:, 2 * pr:2 * pr + 2, :],
                    start=(pr == 0),
                    stop=(pr == CCH // 2 - 1),
                    perf_mode=DR,
                )
            nc.vector.scalar_tensor_tensor(
                out=v8[:, tt // 2, tt % 2, 0:U], in0=aps[:, 0:U],
                scalar=1.0 / 16.0, in1=bv_full, op0=MUL, op1=ADD,
            )

    def emit_pv(tb, tsl):
        aps = apsum.tile([P, 512], F32, tag="aps", name=f"aps{tb}_{tsl}")
        for pr in range(NPR):
            nc.tensor.matmul(
                aps[:, 0:U + 1],
                lhsT=pts_by_tb[tb][pr][:, :, ts(tsl, P)],
                rhs=v8[:, pr, :, 0:U + 1],
                start=(pr == 0),
                stop=(pr == NPR - 1),
                perf_mode=DR,
            )
        return aps

    pending_tr = []

    def emit_drain(tb, tsl, aps):
        rcp = rcp_pool.tile([P, 1], F32, tag="rcp")
        nc.vector.reciprocal(rcp, aps[:, U:U + 1])
        rcps_by_tb.setdefault(tb, []).append(rcp)
        a8 = a_pool.tile([P, U], E4, tag="a8")
        nc.vector.tensor_scalar(out=a8, in0=aps[:, 0:U], scalar1=0.25,
                                scalar2=None, op0=MUL)
        pending_tr.append((a8, tb, tsl))

    def flush_transposes():
        # deferred so the PE never waits on the rcp->a8 DVE chain
        while pending_tr:
            a8, tb, tsl = pending_tr.pop(0)
            # fp8 transpose needs output element step 2 in psum
            tps2 = ypsum.tile([P, UCH, P, 2], E4, tag="yps")
            for uc in range(UCH):
                nc.tensor.transpose(tps2[:, uc, :, 0:1], a8[:, ts(uc, P)],
                                    identity8)
            nc.vector.tensor_copy(
                out=aT8[:, :, ds(tb * TBLK + tsl * P, P)],
                in_=tps2[:, :, :, 0])

    def emit_finish_one(tb, tsl):
        if tsl == 0:
            y_blk_by_tb[tb] = y_pool.tile([P, NTB, C], F32, tag="ysb",
                                          name=f"yblk{tb}")
        yps = ypsum.tile([P, C], F32, tag="yps")
        nc.tensor.matmul(
            yps,
            lhsT=aT8[:, :, ds(tb * TBLK + tsl * P, P)],
            rhs=Wa8,
            start=True,
            stop=True,
            perf_mode=DR,
        )
        nc.vector.scalar_tensor_tensor(
            out=y_blk_by_tb[tb][:, tsl, :], in0=yps, scalar=rcps_by_tb[tb][tsl],
            in1=xba_sb[:, tb * NTB + tsl, :], op0=MUL, op1=ADD,
        )
        if tsl % 2 == 1:
            nc.sync.dma_start(
                out=out[ds(tb * TBLK + (tsl - 1) * P, 2 * P), :].rearrange(
                    "(q p) c -> p q c", p=P),
                in_=y_blk_by_tb[tb][:, tsl - 1:tsl + 1, :])
            if tsl == NTB - 1:
                y_blk_by_tb.pop(tb)

    # scores for tb0 interleaved with the v projection
    for spr in range(NPR):
        emit_score_pair(0, spr)
        emit_v_tiles(2 * spr, 2)

    # --- phase 3: pipelined attention, row-tile bursts ---
    for tb in range(NTB):
        for tsl in range(NTB):
            aps = emit_pv(tb, tsl)
            if tb + 1 < NTB:
                emit_score_pair(tb + 1, 2 * tsl)
                emit_score_pair(tb + 1, 2 * tsl + 1)
            emit_drain(tb, tsl, aps)
            flush_transposes()
            if tb >= 1:
                emit_finish_one(tb - 1, tsl)
            if tb == NTB - 1 and tsl >= 1:
                emit_finish_one(tb, tsl - 1)
        pts_by_tb.pop(tb)
    flush_transposes()
    emit_finish_one(NTB - 1, NTB - 1)

    for pool in (y_pool, rcp_pool, a_pool, p_pool,
                 ypsum, apsum, spsum, persist, consts):
        pool.release()


def _get_nc():
    if "nc" not in _cache:
        nc = bacc.Bacc("TRN2", target_bir_lowering=False, debug=False)
        with tile.TileContext(nc) as tc:
            _build_kernel(tc)
        nc.compile()
        _cache["nc"] = nc
    return _cache["nc"]


def kernel(**inputs):
    nc = _get_nc()
    shared = {k: np.ascontiguousarray(np.asarray(v, dtype=np.float32))
              for k, v in inputs.items() if k != "x"}
    xs = np.ascontiguousarray(np.asarray(inputs["x"], dtype=np.float32))
    in_maps = [dict(shared, x=xs[b]) for b in range(B)]
    res = run_bass_kernel_spmd(nc, in_maps, core_ids=list(range(B)))
    return np.stack([res.results[b]["out"] for b in range(B)], axis=0)
